# revision 1
# baseline (speedup 1.0000x reference)
"""Trainium2 Bass kernel for capsule routing (nn_Capsule).

Reference computation:
    u_hat = einsum('bic,ce->bie', u_vecs, W).reshape(B, I, N, D).transpose(0,2,1,3)
    b = 0
    for r in range(3):
        c = softmax(b, axis=1)                      # over capsules n
        out = squash(einsum('bni,bnid->bnd', c, u_hat))
        if r < 2: b = einsum('bnd,bnid->bni', out, u_hat)
    return out    # (B, N, D)

Key algebraic restructuring (u_hat is never materialized; it is 32 MiB per
core and every use of it factors through u_vecs and W):
    round 0:  c uniform = 1/N  ->  out0 = squash((1/N) * (sum_i u[b,i,:]) @ W)
    logits[b,i,n] = sum_c u[b,i,c] * V[b,c,n],   V[b,c,n] = sum_d W[c,(n,d)] o[b,n,d]
    T[b,n,c]     = sum_i softmax(logits)[b,i,n] * u[b,i,c]
    pre[b,n,d]   = sum_c T[b,n,c] * W[c,(n,d)]   -> out = squash(pre)

Implementation notes:
  - T/pre/squash matmuls use float32r views (single-pass FP22 PE matmul;
    plain fp32 is split into two passes by the compiler, doubling PE time);
    the logits and V matmuls run in bf16, whose full-128-column stationaries
    get the compiler's fast-weight-load (4x LDWEIGHTS) path,
  - squash's rsqrt is a bit-trick seed + 2 fused Newton steps on the Vector
    engine, so the Scalar engine's activation table stays pinned on exp
    (softmax) and never pays the ~2.7us table-set switch,
  - softmax is over the free dim in an (i, n) layout; sums over the
    capsule dim d use ones-vector matmuls on the PE,
  - per-batch logits->softmax->T chains are emitted per b so PE work of
    batch b+1 overlaps the DVE/ACT softmax of batch b.

Sharding: data-parallel over batch, 4 batches per core x 8 cores, W replicated.
"""

import numpy as np
from contextlib import ExitStack

import concourse.bass as bass
import concourse.bacc as bacc
import concourse.tile as tile
from concourse import mybir
from concourse.bass_utils import run_bass_kernel_spmd
from concourse.masks import make_identity

B, I, C = 32, 1024, 256
N, D = 32, 64
ND = N * D
ROUTINGS = 3
EPS = 1e-7
NCORES = 8
BL = B // NCORES  # batches per core
IC = I // 128     # i chunks of 128
CK = C // 128     # c chunks of 128
NB = N * BL       # 128 = (n, b) composite
F32 = mybir.dt.float32
F32R = mybir.dt.float32r
U32 = mybir.dt.uint32
BF16 = mybir.dt.bfloat16
MULT = mybir.AluOpType.mult
AF = mybir.ActivationFunctionType
RSQRT_MAGIC = 0x5F3759DF


def _r(ap):
    """View an fp32 AP as float32r: single-pass (FP22) PE matmul instead of
    the 2-pass fp32 split walrus emits otherwise."""
    return ap.bitcast(F32R)


def _capsule_body(ctx: ExitStack, tc: tile.TileContext, out_ap, u_ap, w_ap):
    nc = tc.nc

    # f32r out-views (PE single-pass matmul inputs) trip the low-precision
    # accumulation guard; the rounding loss (22-bit mantissa) is intentional.
    ctx.enter_context(nc.allow_low_precision(reason="fp32r single-pass matmuls"))

    const = ctx.enter_context(tc.tile_pool(name="const", bufs=1))
    persist = ctx.enter_context(tc.tile_pool(name="persist", bufs=1))
    work = ctx.enter_context(tc.tile_pool(name="work", bufs=4))

    # ---- constants ----
    ident = const.tile([128, 128], F32)
    make_identity(nc, ident[:])
    ones_f = const.tile([128, 2], F32)
    nc.gpsimd.memset(ones_f[:], 1.0)
    ones_col = const.tile([128, 1], F32)
    nc.vector.tensor_copy(out=_r(ones_col[:]), in_=ones_f[:, 0:1])
    ones_row = const.tile([1, 128], F32)
    nc.vector.tensor_copy(out=_r(ones_row[:]), in_=ones_f[0:1, 0:1].to_broadcast([1, 128]))
    magic = const.tile([1, NB], U32)
    nc.gpsimd.memset(magic[:], RSQRT_MAGIC)

    # ---- persistent SBUF tensors ----
    w_sb = persist.tile([128, CK, ND], F32)       # [q, ck, (n,d)]
    wt_sb = persist.tile([64, N, C], BF16)        # [d, n, c] (bf16: V stationary, FWL)
    u_sb = persist.tile([128, BL, IC, C], F32)    # [p, b, ic, c]
    ut_sb = persist.tile([128, BL, CK, I], BF16)  # [q, b, ck, i] (bf16: lg stationary, FWL)
    st_sb = persist.tile([128, CK, BL], F32)      # [q, ck, b]  (column sums of u)

    # ---- load inputs ----
    for ck in range(CK):
        nc.sync.dma_start(out=_r(w_sb[:, ck, :]), in_=_r(w_ap[ck * 128:(ck + 1) * 128, :]))
    for b in range(BL):
        for ic in range(IC):
            nc.sync.dma_start(
                out=_r(u_sb[:, b, ic, :]),
                in_=_r(u_ap[b, ic * 128:(ic + 1) * 128, :]),
            )

    # ---- setup transposes (PE) ----
    with tc.tile_pool(name="ps_setup", bufs=4, space="PSUM") as ps_setup, \
            nc.named_scope("setup"):
        # u blocks: ut[q, b, ck, ic*128:+128] = u[b, i-chunk, c-chunk].T
        for b in range(BL):
            for ck in range(CK):
                for ic in range(IC):
                    ut_ps = ps_setup.tile([128, 128], F32, tag="ut")
                    nc.tensor.transpose(
                        ut_ps[:], u_sb[:, b, ic, ck * 128:(ck + 1) * 128], ident[:]
                    )
                    if (ic + ck) % 2 == 0:
                        nc.vector.tensor_copy(
                            out=ut_sb[:, b, ck, ic * 128:(ic + 1) * 128], in_=ut_ps[:]
                        )
                    else:
                        nc.scalar.copy(
                            out=ut_sb[:, b, ck, ic * 128:(ic + 1) * 128], in_=ut_ps[:]
                        )
        # column sums of u: st[q, ck, b] = sum_i u[b, i, ck-chunk]
        for b in range(BL):
            for ck in range(CK):
                nc.vector.reduce_sum(
                    out=_r(st_sb[:, ck, b:b + 1]),
                    in_=ut_sb[:, b, ck, :],
                    axis=mybir.AxisListType.X,
                )
        # W blocks: wt[d, n, ck*128:+128] = W[ck-chunk, n-block].T
        for ck in range(CK):
            for n in range(N):
                wt_ps = ps_setup.tile([64, 128], F32, tag="wt")
                nc.tensor.transpose(
                    wt_ps[:], w_sb[:, ck, n * 64:(n + 1) * 64], ident[:]
                )
                if n % 2 == 0:
                    nc.vector.tensor_copy(
                        out=wt_sb[0:64, n, ck * 128:(ck + 1) * 128], in_=wt_ps[:]
                    )
                else:
                    nc.scalar.copy(
                        out=wt_sb[0:64, n, ck * 128:(ck + 1) * 128], in_=wt_ps[:]
                    )

    ps = ctx.enter_context(tc.tile_pool(name="ps_main", bufs=1, space="PSUM"))
    ps_pre = ctx.enter_context(tc.tile_pool(name="ps_pre", bufs=1, space="PSUM"))
    ps_t = ctx.enter_context(tc.tile_pool(name="ps_t", bufs=2, space="PSUM"))

    o_sb = None
    for r in range(ROUTINGS):
        if r > 0:
            # V[b][c, n] = sum_d W[c,(n,d)] o[b,n,d]
            with nc.named_scope(f"r{r}_v"):
                v_ps = ps.tile([128, CK, N, BL], F32, tag="v")
                for ck in range(CK):
                    for n in range(N):
                        nc.tensor.matmul(
                            out=v_ps[:, ck, n, :],
                            lhsT=wt_sb[0:64, n, ck * 128:(ck + 1) * 128],
                            rhs=o_sb[:, n * BL:(n + 1) * BL],
                            start=True,
                            stop=True,
                        )
                v_sb = work.tile([128, CK, N, BL], BF16, tag="v_sb")
                for ck in range(CK):
                    nc.scalar.copy(out=v_sb[:, ck], in_=v_ps[:, ck])

            # Per local batch: logits -> softmax -> T -> T^T, pipelined so b+1's
            # PE work overlaps b's DVE/ACT softmax.
            lg_ps = ps.tile([128, BL, IC, N], F32, tag="lg")
            tt_ps = ps.tile([128, CK, N, BL], F32, tag="tt")
            for b in range(BL):
                with nc.named_scope(f"r{r}_lg"):
                    for ic in range(IC):
                        for ck in range(CK):
                            nc.tensor.matmul(
                                out=lg_ps[:, b, ic, :],
                                lhsT=ut_sb[:, b, ck, ic * 128:(ic + 1) * 128],
                                rhs=v_sb[:, ck, :, b],
                                start=(ck == 0),
                                stop=(ck == CK - 1),
                            )
                # softmax over n (free dim; no max-subtraction, logits O(1))
                with nc.named_scope(f"r{r}_sm"):
                    e_sb = work.tile([128, IC, N], F32, tag="e")
                    nc.scalar.activation(out=e_sb[:], in_=lg_ps[:, b], func=AF.Exp)
                    s_sb = work.tile([128, IC], F32, tag="s")
                    nc.vector.reduce_sum(
                        out=s_sb[:], in_=e_sb[:], axis=mybir.AxisListType.X
                    )
                    sr_sb = work.tile([128, IC], F32, tag="sr")
                    nc.vector.reciprocal(out=sr_sb[:], in_=s_sb[:])
                    c_sb = work.tile([128, IC, N], F32, tag="c")
                    nc.vector.tensor_tensor(
                        _r(c_sb[:]),
                        e_sb[:],
                        sr_sb[:, :, None].to_broadcast([128, IC, N]),
                        MULT,
                    )
                # T[b][n, c] = sum_i c[i, n] u[b, i, c]
                with nc.named_scope(f"r{r}_t"):
                    t_ps = ps_t.tile([32, C], F32, tag="t")
                    for ic in range(IC):
                        nc.tensor.matmul(
                            out=t_ps[:],
                            lhsT=_r(c_sb[:, ic, :]),
                            rhs=_r(u_sb[:, b, ic, :]),
                            start=(ic == 0),
                            stop=(ic == IC - 1),
                        )
                    t_sb = work.tile([32, C], F32, tag="t_sb")
                    if b % 2 == 0:
                        nc.scalar.copy(out=t_sb[:], in_=t_ps[:])
                    else:
                        nc.vector.tensor_copy(out=t_sb[:], in_=t_ps[:])
                    for ck in range(CK):
                        nc.tensor.transpose(
                            tt_ps[:, ck, :, b],
                            t_sb[:, ck * 128:(ck + 1) * 128],
                            ident[0:32, 0:32],
                        )
            with nc.named_scope(f"r{r}_t"):
                tt_sb = work.tile([128, CK, N, BL], F32, tag="tt_sb")
                nc.vector.tensor_copy(out=_r(tt_sb[:]), in_=tt_ps[:])

        # ---------- pre[d, (n,b)] per-capsule: pre_n = W_n.T @ T_n ----------
        with nc.named_scope(f"r{r}_pre"):
            pre_ps = ps_pre.tile([64, N, BL], F32, tag="pre")
            for n in range(N):
                for ck in range(CK):
                    rhs = (
                        st_sb[:, ck, :] if r == 0 else tt_sb[:, ck, n, :]
                    )
                    nc.tensor.matmul(
                        out=pre_ps[:, n, :],
                        lhsT=_r(w_sb[:, ck, n * 64:(n + 1) * 64]),
                        rhs=_r(rhs),
                        start=(ck == 0),
                        stop=(ck == CK - 1),
                    )

        # ---------- squash over d (partition dim -> ones-matmul reductions;
        # rsqrt via bit-trick seed + 3 Newton steps, all DVE: keeps the ACT
        # table set pinned to exp for softmax) ----------
        with nc.named_scope(f"r{r}_sq"):
            pre_sb = work.tile([64, NB], F32, tag="pre_sb")
            nc.scalar.copy(out=pre_sb[:], in_=pre_ps[:].rearrange("d n b -> d (n b)"))
            sq_sb = work.tile([64, NB], F32, tag="sq")
            nc.vector.tensor_mul(_r(sq_sb[:]), pre_sb[:], pre_sb[:])
            ss_ps = ps.tile([1, NB], F32, tag="sqps")
            nc.tensor.matmul(
                out=ss_ps[:], lhsT=_r(ones_col[0:64, :]), rhs=_r(sq_sb[:]),
                start=True, stop=True,
            )
            # x = sum/N^2 + eps for r==0 (squash of pre/N), else sum + eps
            x_sb = work.tile([1, NB], F32, tag="x")
            nc.vector.tensor_scalar(
                out=x_sb[:], in0=ss_ps[:],
                scalar1=(1.0 / (N * N) if r == 0 else 1.0), scalar2=EPS,
                op0=MULT, op1=mybir.AluOpType.add,
            )
            # y0 = bitcast(0x5f3759df - (bitcast(x) >> 1))
            yb_sb = work.tile([1, NB], U32, tag="yb")
            nc.vector.tensor_scalar(
                out=yb_sb[:], in0=x_sb[:].bitcast(U32), scalar1=1, scalar2=None,
                op0=mybir.AluOpType.logical_shift_right,
            )
            y_sb = work.tile([1, NB], F32, tag="y")
            nc.vector.tensor_tensor(
                y_sb[:].bitcast(U32), magic[:], yb_sb[:],
                mybir.AluOpType.subtract,
            )
            # Newton: y <- y * (1.5 - 0.5 x y^2), twice (rsqrt rel err ~4e-6)
            for it in range(2):
                t1 = work.tile([1, NB], F32, tag="nt1")
                nc.vector.tensor_mul(t1[:], y_sb[:], y_sb[:])
                nc.vector.scalar_tensor_tensor(
                    out=t1[:], in0=t1[:], scalar=-0.5, in1=x_sb[:],
                    op0=MULT, op1=MULT,
                )
                y2 = work.tile([1, NB], F32, tag="y")
                nc.vector.scalar_tensor_tensor(
                    out=_r(y2[:]), in0=t1[:], scalar=1.5, in1=y_sb[:],
                    op0=mybir.AluOpType.add, op1=MULT,
                )
                y_sb = y2
            if r == 0:
                nc.vector.tensor_scalar_mul(_r(y_sb[:]), y_sb[:], 1.0 / N)
            rnb_ps = ps.tile([64, NB], F32, tag="sqps")
            nc.tensor.matmul(
                out=rnb_ps[:], lhsT=_r(ones_row[0:1, 0:64]), rhs=_r(y_sb[:]),
                start=True, stop=True,
            )
            if r < ROUTINGS - 1:
                o_sb = work.tile([64, NB], BF16, tag="o_bf")
                nc.vector.tensor_tensor(o_sb[:], pre_sb[:], rnb_ps[:], MULT)
            else:
                o_sb = work.tile([64, NB], F32, tag="o")
                nc.vector.tensor_tensor(_r(o_sb[:]), pre_sb[:], rnb_ps[:], MULT)

    # ---------- write out: out[b, n, d] = o[d, (n,b)] ----------
    with nc.named_scope("out"):
        ot_ps = ps.tile([128, 64], F32, tag="sqps")
        nc.tensor.transpose(ot_ps[:], o_sb[:], ident[0:64, 0:64])
        ot_sb = work.tile([128, 64], F32, tag="ot")
        nc.scalar.copy(out=ot_sb[:], in_=ot_ps[:])
        out_nbd = bass.AP(
            tensor=out_ap.tensor,
            offset=out_ap.offset,
            ap=[[D, N], [N * D, BL], [1, D]],
        )
        nc.sync.dma_start(out=out_nbd, in_=ot_sb[:])

def build_program():
    nc = bacc.Bacc("TRN2", target_bir_lowering=False, debug=False)
    u_ap = nc.dram_tensor("u", [BL, I, C], F32, kind="ExternalInput").ap()
    w_ap = nc.dram_tensor("w", [C, ND], F32, kind="ExternalInput").ap()
    out_ap = nc.dram_tensor("out", [BL, N, D], F32, kind="ExternalOutput").ap()
    with tile.TileContext(nc) as tc:
        with ExitStack() as ctx:
            _capsule_body(ctx, tc, out_ap, u_ap, w_ap)
    nc.compile()
    return nc


_NC = None


def kernel(u_vecs: np.ndarray, W: np.ndarray) -> np.ndarray:
    global _NC
    u = np.ascontiguousarray(np.asarray(u_vecs, dtype=np.float32))
    w = np.ascontiguousarray(np.asarray(W, dtype=np.float32))
    assert u.shape == (B, I, C) and w.shape == (C, ND)
    if _NC is None:
        _NC = build_program()
    in_maps = [
        {"u": u[i * BL:(i + 1) * BL], "w": w} for i in range(NCORES)
    ]
    res = run_bass_kernel_spmd(_NC, in_maps, list(range(NCORES)))
    return np.concatenate(
        [res.results[i]["out"] for i in range(NCORES)], axis=0
    )



# revision 15
# speedup vs baseline: 1.2798x; 1.2798x over previous
"""Trainium2 Bass kernel for capsule routing (nn_Capsule).

Reference computation:
    u_hat = einsum('bic,ce->bie', u_vecs, W).reshape(B, I, N, D).transpose(0,2,1,3)
    b = 0
    for r in range(3):
        c = softmax(b, axis=1)                      # over capsules n
        out = squash(einsum('bni,bnid->bnd', c, u_hat))
        if r < 2: b = einsum('bnd,bnid->bni', out, u_hat)
    return out    # (B, N, D)

u_hat (32 MiB/core) is never materialized; routing factors through u and W:
    round 0:  c uniform = 1/N  ->  out0 = squash((1/N) * (sum_i u[b,i,:]) @ W)
    V[b,c,n]  = sum_d W[c,(n,d)] o[b,n,d]
    lg[b,i,n] = sum_c u[b,i,c] V[b,c,n]
    T[b,n,c]  = sum_i softmax(lg)[b,i,n] u[b,i,c]
    pre[b,n,d]= sum_c T[b,n,c] W[c,(n,d)]   -> out = squash(pre)

v2 design (PE-queue time is the kernel critical path):
  - everything on the PE uses bf16 stationaries (4x fast-weight-load) and
    bf16 streams; PSUM accumulation stays fp32,
  - u^T (needed for the lg stationaries) comes from the XBAR DMA transpose
    (dma_start_transpose), not PE transposes,
  - T is computed transposed (T^T[c,(n,b)] with m=c=128, full PE width) so
    it directly forms the stationary of the next matmul,
  - pre is ONE dense streamed chain per round: stationary T^T[c,(n,b)],
    moving W[c,(n,d)] -> out[(n',b),(n,d)]; only the 32 diagonal [4,64]
    blocks are extracted (the redundant columns are free: PE streams one
    column/cycle regardless of how many output rows are used),
  - squash runs on [(n,b),d] with d in the free dim: square, reduce, DVE
    bit-trick rsqrt + 2 Newton steps (ACT table stays pinned on exp),
  - output DMAs directly from [(n,b),d] with a strided DRAM AP.

Sharding: data-parallel over batch, 4 batches per core x 8 cores, W replicated.
"""

import numpy as np
from contextlib import ExitStack

import concourse.bass as bass
import concourse.bacc as bacc
import concourse.tile as tile
from concourse import mybir
from concourse.bass_utils import run_bass_kernel_spmd
from concourse.masks import make_identity

B, I, C = 32, 1024, 256
N, D = 32, 64
ND = N * D
ROUTINGS = 3
EPS = 1e-7
NCORES = 8
BL = B // NCORES  # batches per core
IC = I // 128     # i chunks of 128
CK = C // 128     # c chunks of 128
NB = N * BL       # 128 = (n, b) composite partition dim
PF = 512          # pre free-chunk (one PSUM bank of fp32)
F32 = mybir.dt.float32
U32 = mybir.dt.uint32
BF16 = mybir.dt.bfloat16
MULT = mybir.AluOpType.mult
AF = mybir.ActivationFunctionType
AX = mybir.AxisListType.X
RSQRT_MAGIC = 0x5F3759DF


def _copy(eng, out, in_):
    if hasattr(eng, "tensor_copy"):
        eng.tensor_copy(out=out, in_=in_)
    else:
        eng.copy(out=out, in_=in_)


def _capsule_body(ctx: ExitStack, tc: tile.TileContext, out_ap, u_ap, w_ap):
    nc = tc.nc
    ctx.enter_context(nc.allow_low_precision(reason="bf16 matmul factors"))

    const = ctx.enter_context(tc.tile_pool(name="const", bufs=1))
    persist = ctx.enter_context(tc.tile_pool(name="persist", bufs=1))
    work = ctx.enter_context(tc.tile_pool(name="work", bufs=4))

    # ---- constants ----
    identb = const.tile([128, 128], BF16)
    make_identity(nc, identb[:])
    identf = const.tile([64, 64], F32)
    make_identity(nc, identf[:])
    magic = const.tile([128, 1], U32)
    nc.gpsimd.memset(magic[:], RSQRT_MAGIC)

    # ---- persistent SBUF tensors ----
    w_sb = persist.tile([128, CK, ND], F32)       # [c, ck, (n,d)] raw W
    wb_sb = persist.tile([128, CK, ND], BF16)     # bf16 W (pre stream, wt src)
    wt_sb = persist.tile([64, N, C], BF16)        # [d, n, c] V stationaries
    u_sb = persist.tile([128, BL, IC, C], F32)    # [i, b, ic, c] raw u
    ub_sb = persist.tile([128, BL, IC, C], BF16)  # bf16 u (T stationaries)
    ut_sb = persist.tile([128, BL, CK, I], BF16)  # [c, b, ck, i] lg stationaries
    st_sb = persist.tile([128, CK, BL], F32)      # [c, ck, b] column sums of u
    tsb = persist.tile([128, CK, N, BL], BF16)    # [c, ck, (n,b)] T^T store
    v_sb = persist.tile([128, CK, N, BL], BF16)   # [c, ck, n, b] V store
    ot_sb = persist.tile([64, NB], BF16)          # [d, (n,b)] o^T (V stream)

    # ---- W pipeline: load, cast, transpose ----
    for ck in range(CK):
        nc.sync.dma_start(out=w_sb[:, ck, :], in_=w_ap[ck * 128:(ck + 1) * 128, :])
    for ck in range(CK):
        _copy(nc.vector, wb_sb[:, ck, 0:1024], w_sb[:, ck, 0:1024])
        _copy(nc.scalar, wb_sb[:, ck, 1024:2048], w_sb[:, ck, 1024:2048])
    with tc.tile_pool(name="ps_wt", bufs=4, space="PSUM") as ps_wt, \
            nc.named_scope("setup_w"):
        for ck in range(CK):
            for n in range(N):
                wt_ps = ps_wt.tile([64, 128], BF16, tag="wt")
                nc.tensor.transpose(
                    wt_ps[:], wb_sb[:, ck, n * 64:(n + 1) * 64], identb[:]
                )
                eng = nc.vector if n % 2 == 0 else nc.scalar
                _copy(eng, wt_sb[0:64, n, ck * 128:(ck + 1) * 128], wt_ps[:])

    # ---- u pipeline: load, cast, XBAR-transpose, column sums ----
    with nc.named_scope("setup_u"):
        for b in range(BL):
            for ic in range(IC):
                nc.sync.dma_start(
                    out=u_sb[:, b, ic, :],
                    in_=u_ap[b, ic * 128:(ic + 1) * 128, :],
                )
            for h in range(4):
                eng = (nc.vector, nc.scalar, nc.gpsimd, nc.gpsimd)[h]
                _copy(
                    eng,
                    ub_sb[:, b, 2 * h:2 * h + 2, :],
                    u_sb[:, b, 2 * h:2 * h + 2, :],
                )
            with tc.tile_pool(name=f"ps_ut{b}", bufs=2, space="PSUM") as ps_ut:
                for ck in range(CK):
                    ut_ps = ps_ut.tile([128, I], BF16, tag="ut")
                    for ic in range(IC):
                        nc.tensor.transpose(
                            ut_ps[:, ic * 128:(ic + 1) * 128],
                            ub_sb[:, b, ic, ck * 128:(ck + 1) * 128],
                            identb[:],
                        )
                    eng = nc.vector if ck % 2 == 0 else nc.scalar
                    _copy(eng, ut_sb[:, b, ck, :], ut_ps[:])
            for ck in range(CK):
                nc.vector.reduce_sum(
                    out=st_sb[:, ck, b:b + 1], in_=ut_sb[:, b, ck, :], axis=AX
                )

    ps_lg = ctx.enter_context(tc.tile_pool(name="ps_lg", bufs=2, space="PSUM"))
    ps_v = ctx.enter_context(tc.tile_pool(name="ps_v", bufs=1, space="PSUM"))
    ps_tt = ctx.enter_context(tc.tile_pool(name="ps_tt", bufs=2, space="PSUM"))
    ps_pre = ctx.enter_context(tc.tile_pool(name="ps_pre", bufs=1, space="PSUM"))
    ps_sqt = ctx.enter_context(tc.tile_pool(name="ps_sqt", bufs=1, space="PSUM"))

    o_final = None
    for r in range(ROUTINGS):
        if r == 0:
            # T^T for the uniform round is the column sums, same for every n.
            for ck in range(CK):
                nc.vector.tensor_copy(
                    out=tsb[:, ck],
                    in_=st_sb[:, ck, None, :].to_broadcast([128, N, BL]),
                )
        else:
            # V[b][c, n] = sum_d W[c,(n,d)] o[b,n,d], per-capsule blocks
            with nc.named_scope(f"r{r}_v"):
                v_ps = ps_v.tile([128, CK, N, BL], F32, tag="v")
                for ck in range(CK):
                    for n in range(N):
                        nc.tensor.matmul(
                            out=v_ps[:, ck, n, :],
                            lhsT=wt_sb[0:64, n, ck * 128:(ck + 1) * 128],
                            rhs=ot_sb[0:64, n * BL:(n + 1) * BL],
                            start=True,
                            stop=True,
                        )
                _copy(nc.vector, v_sb[:, 0], v_ps[:, 0])
                _copy(nc.scalar, v_sb[:, 1], v_ps[:, 1])

            # lg -> softmax -> T^T, per-b chains with skewed emission so the
            # PE runs lg(b+1) while DVE/ACT handle softmax(b).
            cs = {}

            def emit_lg(b, r=r):
                lg_ps = ps_lg.tile([128, IC, N], F32, tag="lg")
                with nc.named_scope(f"r{r}_lg"):
                    for ic in range(IC):
                        for ck in range(CK):
                            nc.tensor.matmul(
                                out=lg_ps[:, ic, :],
                                lhsT=ut_sb[:, b, ck, ic * 128:(ic + 1) * 128],
                                rhs=v_sb[:, ck, :, b],
                                start=(ck == 0),
                                stop=(ck == CK - 1),
                            )
                with nc.named_scope(f"r{r}_sm"):
                    e = work.tile([128, IC, N], F32, tag="e")
                    nc.scalar.activation(out=e[:], in_=lg_ps[:], func=AF.Exp)
                    s = work.tile([128, IC], F32, tag="s")
                    nc.vector.reduce_sum(out=s[:], in_=e[:], axis=AX)
                    sr = work.tile([128, IC], F32, tag="sr")
                    nc.vector.reciprocal(out=sr[:], in_=s[:])
                    cb = work.tile([128, IC, N], BF16, tag="c")
                    nc.vector.tensor_tensor(
                        cb[:],
                        e[:],
                        sr[:, :, None].to_broadcast([128, IC, N]),
                        MULT,
                    )
                    cs[b] = cb

            def emit_t(b, r=r):
                with nc.named_scope(f"r{r}_t"):
                    tt = ps_tt.tile([128, CK, N], F32, tag="tt")
                    for ck in range(CK):
                        for ic in range(IC):
                            nc.tensor.matmul(
                                out=tt[:, ck, :],
                                lhsT=ub_sb[:, b, ic, ck * 128:(ck + 1) * 128],
                                rhs=cs[b][:, ic, :],
                                start=(ic == 0),
                                stop=(ic == IC - 1),
                            )
                    _copy(nc.scalar, tsb[:, 0, :, b], tt[:, 0, :])
                    _copy(nc.vector, tsb[:, 1, :, b], tt[:, 1, :])

            emit_lg(0)
            for b in range(1, BL):
                emit_lg(b)
                emit_t(b - 1)
            emit_t(BL - 1)

        # pre[d, (n,b)] per capsule: pre_n = W_n^T @ T_n (bf16 FWL stationary)
        with nc.named_scope(f"r{r}_pre"):
            pre_ps = ps_pre.tile([64, N, BL], F32, tag="prep")
            for n in range(N):
                for ck in range(CK):
                    nc.tensor.matmul(
                        out=pre_ps[:, n, :],
                        lhsT=wb_sb[:, ck, n * 64:(n + 1) * 64],
                        rhs=tsb[:, ck, n, :],
                        start=(ck == 0),
                        stop=(ck == CK - 1),
                    )
            pre64 = work.tile([64, NB], F32, tag="pre64")
            _copy(nc.vector, pre64[:], pre_ps[:].rearrange("d n b -> d (n b)"))
            sqt = ps_sqt.tile([128, D], F32, tag="sqt")
            nc.tensor.transpose(sqt[:], pre64[:], identf[:])
            pre_sb = work.tile([128, D], F32, tag="pre")
            _copy(nc.scalar, pre_sb[:], sqt[:])

        # squash over free dim d; rsqrt = bit-trick seed + 2 Newton steps on
        # DVE (ACT table stays pinned on exp)
        with nc.named_scope(f"r{r}_sq"):
            sq = work.tile([128, D], F32, tag="sq")
            nc.vector.tensor_mul(sq[:], pre_sb[:], pre_sb[:])
            ss = work.tile([128, 1], F32, tag="ss")
            nc.vector.reduce_sum(out=ss[:], in_=sq[:], axis=AX)
            x = work.tile([128, 1], F32, tag="x")
            nc.vector.tensor_scalar(
                out=x[:], in0=ss[:],
                scalar1=(1.0 / (N * N) if r == 0 else 1.0), scalar2=EPS,
                op0=MULT, op1=mybir.AluOpType.add,
            )
            yb = work.tile([128, 1], U32, tag="yb")
            nc.vector.tensor_scalar(
                out=yb[:], in0=x[:].bitcast(U32), scalar1=1, scalar2=None,
                op0=mybir.AluOpType.logical_shift_right,
            )
            y = work.tile([128, 1], F32, tag="y")
            nc.vector.tensor_tensor(
                y[:].bitcast(U32), magic[:], yb[:], mybir.AluOpType.subtract
            )
            for _ in range(2):
                t1 = work.tile([128, 1], F32, tag="nt1")
                nc.vector.tensor_mul(t1[:], y[:], y[:])
                nc.vector.scalar_tensor_tensor(
                    out=t1[:], in0=t1[:], scalar=-0.5, in1=x[:],
                    op0=MULT, op1=MULT,
                )
                y2 = work.tile([128, 1], F32, tag="y")
                nc.vector.scalar_tensor_tensor(
                    out=y2[:], in0=t1[:], scalar=1.5, in1=y[:],
                    op0=mybir.AluOpType.add, op1=MULT,
                )
                y = y2
            if r == 0:
                nc.vector.tensor_scalar_mul(y[:], y[:], 1.0 / N)
            if r < ROUTINGS - 1:
                ob = work.tile([128, D], BF16, tag="ob")
                nc.vector.tensor_tensor(
                    ob[:], pre_sb[:], y[:, 0:1].to_broadcast([128, D]), MULT
                )
                otp = ps_sqt.tile([64, NB], BF16, tag="otp")
                nc.tensor.transpose(otp[:], ob[:], identb[:])
                nc.scalar.copy(out=ot_sb[:], in_=otp[:])
            else:
                o_final = work.tile([128, D], F32, tag="of")
                nc.vector.tensor_tensor(
                    o_final[:], pre_sb[:], y[:, 0:1].to_broadcast([128, D]),
                    MULT,
                )

    # ---- write out: out[b, n, d] <- o[(n,b), d] via strided DRAM AP ----
    with nc.named_scope("out"):
        out_nbd = bass.AP(
            tensor=out_ap.tensor,
            offset=out_ap.offset,
            ap=[[D, N], [N * D, BL], [1, D]],
        )
        nc.sync.dma_start(out=out_nbd, in_=o_final[:])


def build_program():
    nc = bacc.Bacc("TRN2", target_bir_lowering=False, debug=False)
    u_ap = nc.dram_tensor("u", [BL, I, C], F32, kind="ExternalInput").ap()
    w_ap = nc.dram_tensor("w", [C, ND], F32, kind="ExternalInput").ap()
    out_ap = nc.dram_tensor("out", [BL, N, D], F32, kind="ExternalOutput").ap()
    with tile.TileContext(nc) as tc:
        with ExitStack() as ctx:
            _capsule_body(ctx, tc, out_ap, u_ap, w_ap)
    nc.compile()
    return nc


_NC = None


def kernel(u_vecs: np.ndarray, W: np.ndarray) -> np.ndarray:
    global _NC
    u = np.ascontiguousarray(np.asarray(u_vecs, dtype=np.float32))
    w = np.ascontiguousarray(np.asarray(W, dtype=np.float32))
    assert u.shape == (B, I, C) and w.shape == (C, ND)
    if _NC is None:
        _NC = build_program()
    in_maps = [
        {"u": u[i * BL:(i + 1) * BL], "w": w} for i in range(NCORES)
    ]
    res = run_bass_kernel_spmd(_NC, in_maps, list(range(NCORES)))
    return np.concatenate(
        [res.results[i]["out"] for i in range(NCORES)], axis=0
    )
